# revision 21
# baseline (speedup 1.0000x reference)
"""Distributed causal multi-head attention for 8 TRN2 NeuronCores.

Problem: x[4,2048,512], 8 heads, causal. out = Attn(x) @ Wo.T + bo.

Sharding: Megatron-style head split, 2 cores per batch element. Core
(b, hg) computes heads [4*hg, 4*hg+4) of batch b for ALL 2048 queries and
returns the partial O-projection out_hg = att_hg @ Wo[heads hg].T + bo/2;
the host adds the two partials. Every core runs the identical SPMD graph:
8 query panels g=0..7 of 256 rows with EXACT causal k-extent 256*(g+1)
(2*(g+1) k-blocks of 128) - the extents are per-panel constants, identical
on every core, so no mask blocks are wasted and only the last two k-blocks
of each panel (the diagonal) need mask multiplies, with panel-independent
triangular masks.

Device layouts are transposed so that per-query softmax reductions become
matmuls (ones-vector contraction) instead of partition reductions:
  QT[j,q], KT[j,k] from  W.T @ x.T ;  V[k,j] natural;
  S^T[k,q] = KT_head.T @ QT_head (two heads row-packed 0-63 / 64-127);
  P = exp(S^T/8) on ScalarE, diagonal blocks masked by multiply;
  o^T[d,q] accumulated over k-blocks with the two heads col-packed; den
  via a ones column appended to V; out[q,:] = attT.T @ Wo.T + bo/2.

Pipelining: PV batches, softmax-normalize chains and O-projections are
emitted DEFERRED - drained between later score batches - so the in-order
PE queue always has ready matmuls and the HAM clock gate stays warm.
Input x^T is DMAed in four 512-column chunks with the QKV projections
interleaved per chunk, so the PE starts ~4us in instead of waiting for
the full input load.
"""

import os
import sys

import numpy as np

sys.path.insert(0, "/opt/trn_rl_repo")

import concourse.bass as bass  # noqa: E402
import concourse.mybir as mybir  # noqa: E402
from concourse import bacc  # noqa: E402
from concourse.tile import TileContext  # noqa: E402

P = 128
D = 512
S = 2048
H = 8
HL = 4  # heads per core
DL = HL * 64  # local projection width (256)
DH = 64
NPANEL = 8
QP = 256  # query rows per panel
NEG = -1.0e9
SCALE = 0.125  # 1/sqrt(DH)

MMDT_NAME = os.environ.get("KERNEL_MMDT", "bf16")  # bf16 | f32r | f32

f32 = mybir.dt.float32
Exp = mybir.ActivationFunctionType.Exp
add_op = mybir.AluOpType.add
mult_op = mybir.AluOpType.mult

# Matmul-operand dtype. bf16 runs 4x faster than f32 on PE (1 cyc/row) and
# halves SBUF/DMA for the big tensors; PSUM accumulation stays f32.
MMDT = {"bf16": mybir.dt.bfloat16, "f32r": mybir.dt.float32r, "f32": f32}[MMDT_NAME]


def build():
    # Bacc (not Bass): its compile() pipeline runs generate_event_semaphores,
    # which splits multi-wait instructions to satisfy the 1-wait-per-
    # instruction hardware limit.
    nc = bacc.Bacc()

    # x^T packed partition-major by the host: [p, kchunk, dblock, 512] so
    # each chunk DMA is one 4KB-contiguous packet per partition.
    xT = nc.declare_dram_parameter("xT", [P, 4, 4, 512], MMDT, isOutput=False)
    wqT = nc.declare_dram_parameter("wqT", [P, 4, DL], MMDT, isOutput=False)
    wkT = nc.declare_dram_parameter("wkT", [P, 4, DL], MMDT, isOutput=False)
    wvT = nc.declare_dram_parameter("wvT", [P, 4, DL], MMDT, isOutput=False)
    woT = nc.declare_dram_parameter("woT", [DH, HL, D], MMDT, isOutput=False)
    bq = nc.declare_dram_parameter("bq", [P, 2], f32, isOutput=False)
    bk = nc.declare_dram_parameter("bk", [P, 2], f32, isOutput=False)
    # V bias is folded into bo on the host: (sum_k p_k (v_k + bv))/den =
    # PV/den + bv, and the constant bv row maps through Wo to a bias term.
    bo_bc = nc.declare_dram_parameter("bo_bc", [P, D], f32, isOutput=False)
    mask = nc.declare_dram_parameter("mask", [P, 2, QP], MMDT, isOutput=False)
    ones64 = nc.declare_dram_parameter("ones64", [1, DH], MMDT, isOutput=False)
    out = nc.declare_dram_parameter("out", [S, D], f32, isOutput=True)

    with nc.allow_low_precision(reason="bf16 matmul operands"), TileContext(nc) as tc:
        with (
            tc.tile_pool(name="big", bufs=1) as bpool,
            tc.tile_pool(name="attp", bufs=2) as apool,
            tc.tile_pool(name="work", bufs=4) as wpool,
            tc.tile_pool(name="osb", bufs=2) as opool,
            tc.tile_pool(name="ps_proj", bufs=2, space="PSUM") as ps_proj,
            tc.tile_pool(name="ps_s", bufs=2, space="PSUM") as ps_s,
            tc.tile_pool(name="ps_ot", bufs=2, space="PSUM") as ps_ot,
        ):
            # ---- persistent SBUF tensors ----
            xT_sb = bpool.tile([P, 4, 4, 512], MMDT, tag="xT")  # [p, kc, db, q]
            # K^T/Q^T stored head-major on partitions 0-63: matmuls with
            # operands at SBUF base-partition 64 (auto row tile_position)
            # are device-fatal (NRT_EXEC_UNIT_UNRECOVERABLE), so every
            # attention matmul must run at partition base 0.
            kT_sb = bpool.tile([DH, HL, S], MMDT, tag="kT")
            qT_sb = bpool.tile([DH, HL, S], MMDT, tag="qT")
            v_sb = bpool.tile([P, S // P, HL, DH + 1], MMDT, tag="v")
            # The sync HWDGE queue starts pulling first; feed it the first
            # projection's critical chain (wk, x chunk 0) ahead of everything,
            # then the rest of x. Small/late-need tensors ride the scalar
            # queue in parallel.
            w_sb = {}
            for name, prm in (("wq", wqT), ("wk", wkT), ("wv", wvT)):
                w_sb[name] = bpool.tile([P, 4, DL], MMDT, tag=name, name=name)
            wo_sb = bpool.tile([DH, HL, D], MMDT, tag="wo")
            bq_sb = bpool.tile([P, 2], f32, tag="bq")
            bk_sb = bpool.tile([P, 2], f32, tag="bk")
            bo_sb = bpool.tile([P, D], f32, tag="bo")
            mask_sb = bpool.tile([P, 2, QP], MMDT, tag="mask")
            o64_sb = bpool.tile([1, DH], MMDT, tag="o64")

            # sync queue, dependency order: the first K-proj accumulation
            # needs wk[db] + x[chunk0, db] piecewise, so split those by db
            for db in range(4):
                nc.sync.dma_start(out=w_sb["wk"][:, db, :], in_=wkT[:, db, :])
            for db in range(4):
                nc.sync.dma_start(out=xT_sb[:, 0, db, :], in_=xT[:, 0, db, :])
            nc.sync.dma_start(out=w_sb["wq"][:], in_=wqT[:])
            nc.sync.dma_start(out=xT_sb[:, 1], in_=xT[:, 1])
            nc.sync.dma_start(out=w_sb["wv"][:], in_=wvT[:])
            nc.sync.dma_start(out=xT_sb[:, 2], in_=xT[:, 2])
            nc.sync.dma_start(out=xT_sb[:, 3], in_=xT[:, 3])
            # scalar queue: biases (needed right after the first proj
            # matmuls), mask, O-proj weights
            nc.scalar.dma_start(out=bk_sb[:], in_=bk[:])
            nc.scalar.dma_start(out=bq_sb[:], in_=bq[:])
            nc.scalar.dma_start(out=mask_sb[:], in_=mask[:])
            nc.scalar.dma_start(out=o64_sb[:], in_=ones64[:])
            nc.scalar.dma_start(out=wo_sb[:], in_=woT[:])
            nc.scalar.dma_start(out=bo_sb[:], in_=bo_bc[:])
            nc.vector.memset(v_sb[:, :, :, DH : DH + 1], 1.0)

            # Deferred-emission queue: closures that emit PV batches,
            # normalize chains and O-projections AFTER later score batches,
            # keeping the in-order PE queue fed.
            deferred = []

            def drain(n):
                for _ in range(min(n, len(deferred))):
                    deferred.pop(0)()

            # ---- projections for one 512-column k-chunk ----
            # PSUM->SBUF copies ride the idle ACT engine for the first two
            # chunks (before the exp stream saturates it), DVE afterwards.
            def emit_proj_chunk(kc):
                use_act = kc <= 1
                # KT/QT[j,k] = sum_d W.T[d,j] * xT[d,k]  (+b per-partition j)
                for wname, bsb, dst in (("wk", bk_sb, kT_sb), ("wq", bq_sb, qT_sb)):
                    for jb in range(2):
                        ps = ps_proj.tile([P, 512], f32, tag="p512")
                        for db in range(4):
                            nc.tensor.matmul(
                                ps[:],
                                lhsT=w_sb[wname][:, db, jb * P : (jb + 1) * P],
                                rhs=xT_sb[:, kc, db, :],
                                start=(db == 0),
                                stop=(db == 3),
                            )
                        stg = wpool.tile([P, 512], MMDT, tag="stage")
                        if use_act:
                            nc.scalar.activation(
                                stg[:], ps[:],
                                mybir.ActivationFunctionType.Identity,
                                bias=bsb[:, jb : jb + 1],
                            )
                        else:
                            nc.vector.tensor_tensor(
                                stg[:],
                                ps[:],
                                bsb[:, jb : jb + 1].to_broadcast([P, 512]),
                                add_op,
                            )
                        for hh in range(2):
                            nc.sync.dma_start(
                                out=dst[:, 2 * jb + hh, kc * 512 : (kc + 1) * 512],
                                in_=stg[hh * DH : (hh + 1) * DH, :],
                            )
                        drain(1)
                # V[k,j] = sum_d xT[d,k] * Wv.T[d,j] (bias folded into bo); a
                # ones column per head (memset above) makes P.V also yield the
                # softmax denominator in psum row DH for free
                for kb in range(4 * kc, 4 * kc + 4):
                    ps = ps_proj.tile([P, 512], f32, tag="p512")
                    psv = ps[:, 0:DL]
                    kq = (kb % 4) * P
                    for db in range(4):
                        nc.tensor.matmul(
                            psv,
                            lhsT=xT_sb[:, kb // 4, db, kq : kq + P],
                            rhs=w_sb["wv"][:, db, :],
                            start=(db == 0),
                            stop=(db == 3),
                        )
                    vdst = v_sb[:, kb, :, 0:DH]
                    vsrc = psv.rearrange("p (h d) -> p h d", h=HL)
                    if use_act:
                        nc.scalar.activation(
                            vdst, vsrc, mybir.ActivationFunctionType.Copy
                        )
                    else:
                        nc.vector.tensor_copy(out=vdst, in_=vsrc)
                    drain(1)

            # ---- attention stage (panel pair kc, head h) ----
            # The two 256-row panels A=2kc, B=2kc+1 share all k-blocks below
            # A's exact extent: those score/PV matmuls run once with free dim
            # 512 spanning both panels' queries. Only B's two diagonal blocks
            # run at free dim 256. Same computed elements, ~45% fewer PE
            # instructions (shared LDWEIGHTS, wider streams).
            def make_pv(kb0, pT, ot_ps, h, w, col0, start, stop):
                def emit():
                    for kbi in range(2):
                        nc.tensor.matmul(
                            ot_ps[:, col0 : col0 + w],
                            lhsT=v_sb[:, kb0 + kbi, h, :],
                            rhs=pT[:, kbi, 0:w],
                            start=(start and kbi == 0),
                            stop=(stop and kbi == 1),
                        )

                return emit

            def make_norm(ot_ps, attT_sb, h):
                def emit():
                    # custom-DVE reciprocal cannot read PSUM (silent garbage)
                    # - copy den to SBUF first
                    den_sb = wpool.tile([1, 512], f32, tag="den_sb")
                    nc.vector.tensor_copy(out=den_sb[:], in_=ot_ps[DH : DH + 1, :])
                    rden_f = wpool.tile([1, 512], f32, tag="rden_f")
                    nc.vector.reciprocal_approx_fast(out=rden_f[:], in_=den_sb[:])
                    rden = wpool.tile([1, 512], MMDT, tag="rden")
                    nc.vector.tensor_copy(out=rden[:], in_=rden_f[:])
                    bc_t = ps_proj.tile([P, 512], f32, tag="p512", name="bc_t")
                    nc.tensor.matmul(
                        bc_t[0:DH, :], lhsT=o64_sb[:], rhs=rden[:],
                        start=True, stop=True,
                    )
                    bc_sb = wpool.tile([DH, 512], f32, tag="bc_sb")
                    nc.vector.tensor_copy(out=bc_sb[:], in_=bc_t[0:DH, :])
                    nc.vector.tensor_mul(
                        out=attT_sb[:, h, :], in0=ot_ps[0:DH, :], in1=bc_sb[:]
                    )

                return emit

            def make_oproj(kc, attT_sb, qs):
                def emit():
                    ps = ps_proj.tile([P, 512], f32, tag="p512", name="ps_o")
                    for hb in range(HL):
                        nc.tensor.matmul(
                            ps[:],
                            lhsT=attT_sb[:, hb, qs * P : (qs + 1) * P],
                            rhs=wo_sb[:, hb, :],
                            start=(hb == 0),
                            stop=(hb == HL - 1),
                        )
                    osb = opool.tile([P, D], f32, tag="osb")
                    nc.vector.tensor_tensor(osb[:], ps[:], bo_sb[:], add_op)
                    nc.sync.dma_start(
                        out=out[kc * 512 + qs * P : kc * 512 + (qs + 1) * P, :],
                        in_=osb[:],
                    )

                return emit

            def emit_stage(kc, h, attT_sb):
                q0 = 512 * kc
                nshared = 2 * kc + 1  # batches of 2 k-blocks shared by A and B
                ot_ps = ps_ot.tile([DH + 1, 512], f32, tag="ot")
                for bb in range(nshared + 1):
                    final = bb == nshared  # B's diagonal blocks, B's cols only
                    w = QP if final else 512
                    qs0 = q0 + QP if final else q0
                    s_ps = ps_s.tile([P, 2, 512], f32, tag="s")
                    for kbi in range(2):
                        kb = 2 * bb + kbi
                        nc.tensor.matmul(
                            s_ps[:, kbi, 0:w],
                            lhsT=kT_sb[:, h, kb * P : (kb + 1) * P],
                            rhs=qT_sb[:, h, qs0 : qs0 + w],
                            start=True,
                            stop=True,
                        )
                    drain(2)
                    pT = wpool.tile([P, 2, 512], MMDT, tag="pT", bufs=6)
                    nc.scalar.activation(pT[:, :, 0:w], s_ps[:, :, 0:w], Exp, scale=SCALE)
                    if bb == 2 * kc or final:
                        # diagonal triangles: batch 2kc holds panel A's two
                        # diagonal blocks (A's query cols 0:256); the final
                        # batch holds B's (stored at tile cols 0:256)
                        nc.vector.tensor_tensor(
                            pT[:, :, 0:QP], pT[:, :, 0:QP], mask_sb[:], mult_op
                        )
                    deferred.append(
                        make_pv(2 * bb, pT, ot_ps, h, w, QP if final else 0,
                                bb == 0, final)
                    )
                deferred.append(make_norm(ot_ps, attT_sb, h))

            # ---- main schedule: projections run one chunk ahead of the
            # attention pair they feed, so kT/qT staging DMAs complete
            # during the previous attention block and never block scores
            attT = {}
            emit_proj_chunk(0)
            for kc in range(4):
                if kc + 1 < 4:
                    emit_proj_chunk(kc + 1)
                attT[kc] = apool.tile([DH, HL, 512], MMDT, tag="attT", name="attT")
                for h in range(HL):
                    emit_stage(kc, h, attT[kc])
                for qs in range(4):
                    deferred.append(make_oproj(kc, attT[kc], qs))
            drain(len(deferred))
    return nc


_NC = None


def _get_nc():
    global _NC
    if _NC is None:
        _NC = build()
        # run_bass_via_pjrt does not finalize; Bacc.finalize runs the compile
        # passes (register allocation, event-semaphore wait splitting).
        _NC.finalize()
    return _NC


def _make_mask(mmnp):
    r = np.arange(P)[:, None]
    c = np.arange(QP)[None, :]
    m = np.empty((P, 2, QP), np.float32)
    m[:, 0, :] = np.where(r <= c, 1.0, 0.0)
    m[:, 1, :] = np.where(r + 128 <= c, 1.0, 0.0)
    return m.astype(mmnp)


def _in_maps(inputs):
    mmnp = mybir.dt.np(MMDT)
    x = np.asarray(inputs["x"], np.float32)
    wqT = np.asarray(inputs["W_Q_w"], np.float32).T  # [D, D] = [d, j]
    wkT = np.asarray(inputs["W_K_w"], np.float32).T
    wvT = np.asarray(inputs["W_V_w"], np.float32).T
    woT = np.asarray(inputs["W_O_w"], np.float32).T  # [j, n]
    bq_f = np.asarray(inputs["W_Q_b"], np.float32)
    bk_f = np.asarray(inputs["W_K_b"], np.float32)
    bv_f = np.asarray(inputs["W_V_b"], np.float32)
    bo_f = np.asarray(inputs["W_O_b"], np.float32)
    mask = _make_mask(mmnp)
    ones64 = np.ones((1, DH), mmnp)

    def pack_x(xb):
        # [D, S] -> [p, kchunk, dblock, 512]: row d = db*128+p, col q =
        # kc*512+qc; contiguous 4KB per partition per chunk for fat DMAs
        t = xb.T.reshape(4, P, 4, 512)  # [db, p, kc, qc]
        return np.ascontiguousarray(t.transpose(1, 2, 0, 3)).astype(mmnp)

    def pack_w(wT, j0):
        # [D, DL] -> [p, db, j]: row d = db*128+p
        t = wT[:, j0 : j0 + DL].reshape(4, P, DL)
        return np.ascontiguousarray(t).transpose(1, 0, 2).copy().astype(mmnp)

    xTs = [pack_x(x[b]) for b in range(4)]
    per_hg = []
    for hg in range(2):
        j0 = hg * DL
        per_hg.append(
            {
                "wqT": pack_w(wqT, j0),
                "wkT": pack_w(wkT, j0),
                "wvT": pack_w(wvT, j0),
                "woT": np.ascontiguousarray(
                    woT[j0 : j0 + DL, :].reshape(HL, DH, D).transpose(1, 0, 2)
                ).astype(mmnp),
                "bq": np.ascontiguousarray(bq_f[j0 : j0 + DL].reshape(2, P).T),
                "bk": np.ascontiguousarray(bk_f[j0 : j0 + DL].reshape(2, P).T),
                # V bias folded through Wo, plus this core's half of bo;
                # the host sums the two partials per batch
                "bo_bc": np.ascontiguousarray(
                    np.broadcast_to(
                        0.5 * bo_f + bv_f[j0 : j0 + DL] @ woT[j0 : j0 + DL, :],
                        (P, D),
                    )
                ),
            }
        )
    in_maps = []
    for core in range(8):
        b, hg = core // 2, core % 2
        m = {"xT": xTs[b], "mask": mask, "ones64": ones64}
        m.update(per_hg[hg])
        in_maps.append(m)
    return in_maps


def _assemble(results, B=4):
    out = np.empty((B, S, D), np.float32)
    for b in range(B):
        out[b] = results[2 * b]["out"]
        out[b] += results[2 * b + 1]["out"]
    return out


def run(inputs, trace=False, **kw):
    from concourse.bass_utils import run_bass_kernel_spmd

    res = run_bass_kernel_spmd(
        _get_nc(), _in_maps(inputs), core_ids=list(range(8)), trace=trace, **kw
    )
    return _assemble(res.results), res


def kernel(**inputs):
    out, _ = run(inputs, trace=False)
    return out
